# revision 74
# baseline (speedup 1.0000x reference)
"""Trainium2 Bass kernel for nn_BasicTransformerBlock (self-attn + cross-attn + GEGLU).

Sharding: data-parallel over the 2048 tokens (256 per core, 8 cores, no
collectives). K/V for self-attention are computed replicated on every core.
Each core's xT input is rotated so its own 256 tokens are columns 0:256 —
softmax over keys is permutation-invariant, so K/V order doesn't matter,
and Q/LN-own can slice the full-sequence LN output directly.

On-chip layout is feature-major throughout ([feature(part), token(free)]).
Host pre-packs weights as bf16 W.T (C-contiguous [in, out]) and pre-transposes
x / context, so the device does zero transposes/casts and all DMAs are
contiguous. Weight/projection matmuls run in bf16 (fp32 PSUM accumulate);
LayerNorm statistics run in float32r off the fp32 residual stream; rstd is
exp(-0.5*ln(var+eps)) on ScalarE (same activation-table set as attention exp).

Softmax (attn1): scores keys-on-partitions ([keys, q]); exp batched 4 key-tiles
per activation; denominators from an appended ones-column in V (row 64 of the
AV accumulation); per-head reciprocal hides under the exp stream.
Softmax (attn2): denominators via e.T@ones matmuls into a [q,*] PSUM tile,
one batched reciprocal per 8-head group, PE-transpose + selector-broadcast
to get per-head [1,q] rows back.
"""

import numpy as np
import ml_dtypes

import concourse.bass as bass
import concourse.mybir as mybir
import concourse.tile as tile
from concourse import bacc
from concourse.bass_utils import run_bass_kernel_spmd

F32 = mybir.dt.float32
F32R = mybir.dt.float32r
BF16 = mybir.dt.bfloat16
FP8 = mybir.dt.float8e4
DRow = mybir.MatmulPerfMode.DoubleRow
AF = mybir.ActivationFunctionType
OP = mybir.AluOpType

AE = 8.0      # (unused) attn1 exp output fp8 scale
AVS = 32.0    # V fp8 scale (folded out via the reciprocal broadcast row)
CL = 32.0     # LN1 output fp8 scale (folded into the rstd broadcast row)

P = 128
N, D = 2048, 1024
H, DH = 16, 64
CN, CD = 77, 768
FF = 4096
EPS = 1e-5
SCALE = DH ** -0.5
NCORES = 8
TO = N // NCORES          # 256 tokens owned per core
DT = D // P               # 8 feature tiles
CT = CD // P              # 6 context-feature tiles
NKT = N // P              # 16 key tiles
FT = FF // P              # 32 ffn-inner tiles


def _ln_feature_major(nc, lnp, sbp, consts, src_of, dst_of, n_dt, tn, chunk,
                      post_cb=None, bscale_row=None, alt_mult=False):
    """Un-affine LayerNorm over feature-major f32r data.

    bscale_row: optional [1,P] const row used as the rstd broadcast lhsT;
    a row of value c makes the output c*(x-mu)*rstd (fp8 pre-scale).
    """
    ones_col, ones_row, eps_t = consts
    inv_d = 1.0 / (n_dt * P)
    for tci in range(tn // chunk):
        srcs = [src_of(dt, tci) for dt in range(n_dt)]   # f32r tiles
        sum_ps = lnp.tile([1, chunk], F32, tag="ln_sum", bufs=2)
        for dt in range(n_dt):
            nc.tensor.matmul(sum_ps, ones_col, srcs[dt],
                             start=(dt == 0), stop=(dt == n_dt - 1))
        sumsq_ps = lnp.tile([1, chunk], F32, tag="ln_sumsq", bufs=2)
        for dt in range(n_dt):
            sq_t = sbp.tile([P, chunk], F32R, tag="ln_sq", bufs=3)
            if dt % 2:
                nc.scalar.activation(sq_t, srcs[dt].bitcast(F32), AF.Square)
            else:
                nc.gpsimd.tensor_tensor(out=sq_t, in0=srcs[dt].bitcast(F32),
                                        in1=srcs[dt].bitcast(F32), op=OP.mult)
            nc.tensor.matmul(sumsq_ps, ones_col, sq_t,
                             start=(dt == 0), stop=(dt == n_dt - 1))
        mu_row = sbp.tile([1, chunk], F32R, tag="ln_mu", bufs=2)
        nc.scalar.mul(out=mu_row, in_=sum_ps, mul=inv_d)
        var_row = sbp.tile([1, chunk], F32, tag="ln_var", bufs=2)
        nc.scalar.mul(out=var_row, in_=sumsq_ps, mul=inv_d)
        musq = sbp.tile([1, chunk], F32, tag="ln_musq", bufs=2)
        nc.vector.tensor_mul(out=musq, in0=mu_row.bitcast(F32),
                             in1=mu_row.bitcast(F32))
        nc.vector.tensor_tensor(out=var_row, in0=var_row, in1=musq,
                                op=OP.subtract)
        # rstd = (var+eps)^-0.5 via exp(-0.5*ln(var+eps)): stays in the
        # natural_log/exp activation-table family, no DVE reciprocal.
        lnv_row = sbp.tile([1, chunk], F32, tag="ln_lnv", bufs=2)
        nc.scalar.activation(lnv_row, var_row, AF.Ln, bias=eps_t)
        rstd_row = sbp.tile([1, chunk], F32R, tag="ln_rstd", bufs=2)
        nc.scalar.activation(rstd_row, lnv_row, AF.Exp, scale=-0.5)
        mu_b = lnp.tile([P, chunk], F32, tag="ln_mub", bufs=1)
        nc.tensor.matmul(mu_b, ones_row, mu_row, start=True, stop=True)
        rstd_b = lnp.tile([P, chunk], F32, tag="ln_rstdb", bufs=1)
        nc.tensor.matmul(rstd_b,
                         ones_row if bscale_row is None else bscale_row,
                         rstd_row, start=True, stop=True)
        mu_s = sbp.tile([P, chunk], F32, tag="ln_mus", bufs=2)
        nc.scalar.copy(out=mu_s, in_=mu_b)
        rstd_s = None
        if alt_mult:
            # SBUF copy so the Pool engine (no PSUM port) can do half
            # the normalize multiplies.
            rstd_s = sbp.tile([P, chunk], F32, tag="ln_rstds", bufs=2)
            nc.scalar.copy(out=rstd_s, in_=rstd_b)
        for dt in range(n_dt):
            tmp = sbp.tile([P, chunk], F32, tag="ln_tmp", bufs=3)
            eng = nc.gpsimd if dt % 2 else nc.vector
            src_in = mu_s if dt % 2 else mu_b
            eng.tensor_tensor(out=tmp, in0=srcs[dt].bitcast(F32),
                              in1=src_in, op=OP.subtract)
            if alt_mult and dt % 2:
                nc.gpsimd.tensor_tensor(out=dst_of(dt, tci), in0=tmp,
                                        in1=rstd_s, op=OP.mult)
            else:
                nc.vector.tensor_tensor(out=dst_of(dt, tci), in0=tmp,
                                        in1=rstd_b, op=OP.mult)
        if post_cb is not None:
            post_cb(tci)


def build(flags, qexps=None):
    has_qkv1b, has_bo1, has_q2b, has_bo2, has_gegb, has_ffb = flags
    fp8_kv = qexps is not None and not has_qkv1b
    nc = bacc.Bacc()

    wdt1 = FP8 if fp8_kv else BF16
    xT = nc.dram_tensor("xT", [D, N], F32R, kind="ExternalInput")
    ctxT = nc.dram_tensor("ctxT", [CD, CN], BF16, kind="ExternalInput")
    wq1T = nc.dram_tensor("wq1T", [D, D], wdt1, kind="ExternalInput")
    wk1T = nc.dram_tensor("wk1T", [D, D], wdt1, kind="ExternalInput")
    wv1T = nc.dram_tensor("wv1T", [D, D], wdt1, kind="ExternalInput")
    wo1T = nc.dram_tensor("wo1T", [D, D], BF16, kind="ExternalInput")
    wq2T = nc.dram_tensor("wq2T", [D, D], BF16, kind="ExternalInput")
    wk2T = nc.dram_tensor("wk2T", [CD, D], BF16, kind="ExternalInput")
    wv2T = nc.dram_tensor("wv2T", [CD, D], BF16, kind="ExternalInput")
    wo2T = nc.dram_tensor("wo2T", [D, D], BF16, kind="ExternalInput")
    wgT = nc.dram_tensor("wgT", [D, 2 * FF], BF16, kind="ExternalInput")
    wfT = nc.dram_tensor("wfT", [FF, D], BF16, kind="ExternalInput")
    onesc = nc.dram_tensor("onesc", [P, 1], F32R, kind="ExternalInput")
    onescb = nc.dram_tensor("onescb", [P, 1], BF16, kind="ExternalInput")
    onesr = nc.dram_tensor("onesr", [1, P], F32R, kind="ExternalInput")
    invavr = nc.dram_tensor("invavr", [1, P], F32R, kind="ExternalInput")
    onesb = nc.dram_tensor("onesb", [1, 512], BF16, kind="ExternalInput")
    selm = nc.dram_tensor("selm", [16, 1024], BF16, kind="ExternalInput")
    identb = nc.dram_tensor("identb", [P, P], BF16, kind="ExternalInput")
    bias_rows = {}
    if has_qkv1b:
        for nm in ("bq1", "bk1", "bv1"):
            bias_rows[nm] = nc.dram_tensor(nm, [1, D], BF16, kind="ExternalInput")
    if has_bo1:
        bias_rows["bo1"] = nc.dram_tensor("bo1", [1, D], BF16, kind="ExternalInput")
    if has_q2b:
        bias_rows["bq2"] = nc.dram_tensor("bq2", [1, D], BF16, kind="ExternalInput")
    if has_bo2:
        bias_rows["bo2"] = nc.dram_tensor("bo2", [1, D], BF16, kind="ExternalInput")
    if has_gegb:
        bias_rows["bgeg"] = nc.dram_tensor("bgeg", [1, 2 * FF], BF16,
                                           kind="ExternalInput")
    if has_ffb:
        bias_rows["bff"] = nc.dram_tensor("bff", [1, D], BF16, kind="ExternalInput")
    yT = nc.dram_tensor("yT", [D, TO], F32R, kind="ExternalOutput")

    xT_v = xT.rearrange("(dt p) t -> dt p t", p=P)
    ctxT_v = ctxT.rearrange("(ct p) t -> ct p t", p=P)
    yT_v = yT.rearrange("(dt p) t -> p dt t", p=P)

    def wview(w):
        return w.rearrange("(it p) o -> p it o", p=P)

    with tile.TileContext(nc) as tc:
        with tc.tile_pool(name="consts", bufs=1) as cpool, \
             tc.tile_pool(name="pers", bufs=1) as pers, \
             tc.tile_pool(name="wmain", bufs=1) as wmain:

            ones_col = cpool.tile([P, 1], F32R)
            nc.sync.dma_start(ones_col, onesc[:])
            ones_colb = cpool.tile([P, 1], BF16)
            nc.sync.dma_start(ones_colb, onescb[:])
            ones_row = cpool.tile([1, P], F32R)
            nc.sync.dma_start(ones_row, onesr[:])
            ones_b = cpool.tile([1, 512], BF16)
            nc.sync.dma_start(ones_b, onesb[:])
            invav_row = cpool.tile([1, P], F32R)
            nc.sync.dma_start(invav_row, invavr[:])
            cl_f32 = cpool.tile([1, P], F32)
            nc.vector.memset(cl_f32, CL)
            cl_row = cl_f32.bitcast(F32R)
            sel_sb = cpool.tile([16, 1024], BF16)
            nc.sync.dma_start(sel_sb, selm[:])
            ident_sb = cpool.tile([P, P], BF16)
            nc.sync.dma_start(ident_sb, identb[:])
            eps_t = cpool.tile([1, 1], F32)
            nc.vector.memset(eps_t, EPS)
            consts = (ones_col, ones_row, eps_t)

            bias_sb = {}
            for nm, t in bias_rows.items():
                bt = cpool.tile([1, t.shape[1]], BF16, tag=f"bias_{nm}")
                nc.sync.dma_start(bt, t[:])
                bias_sb[nm] = bt

            def proj_feature_major(pp, w_sb, act, out_cb, n_in, n_tok,
                                   bias=None, tag="pp256"):
                """out[oc] = sum_it w.T @ act; out_cb(oc, psum)."""
                for oc in range(DT):
                    ps = pp.tile([P, n_tok], F32, tag=tag, bufs=2)
                    for it in range(n_in):
                        nc.tensor.matmul(ps, w_sb[:, it, oc * P:(oc + 1) * P],
                                         act[:, it, :],
                                         start=(it == 0),
                                         stop=(it == n_in - 1 and bias is None))
                    if bias is not None:
                        nc.tensor.matmul(ps, bias[:, oc * P:(oc + 1) * P],
                                         ones_b[:, :n_tok], start=False,
                                         stop=True)
                    out_cb(oc, ps)

            x_ownT = pers.tile([P, DT, TO], F32R)      # residual stream (own)
            # (loaded at phase-C start: only phD's residual needs it, and the
            # early DMA bandwidth is critical for the LN1 lead-in)

            # cross-attn K2/V2 depend only on the context: computed early in
            # phase B so they overlap everything up to phase E.
            K2_sb = pers.tile([P, DT, CN], BF16)
            V2_sb = pers.tile([P, H, DH], BF16)

            # ========== attn1 scope: phases A-D ==========
            with tc.tile_pool(name="c1", bufs=1) as c1:
                O_sb = c1.tile([P, DT, TO], BF16)
                K_sb = c1.tile([P, DT, N], BF16)
                V_sb = c1.tile([P, NKT, H, 65], FP8)
                Q_sb = c1.tile([P, DT, TO], BF16)

                with tc.tile_pool(name="c2", bufs=1) as c2:
                    ln1T = c2.tile([P, DT, N], FP8 if fp8_kv else BF16)
                    if fp8_kv:
                        swq, swk, swv = qexps
                        dsq = 1.0 / (CL * swq)
                        dsk = 1.0 / (CL * swk)
                        dsv = AVS / (CL * swv)

                    # ----- Phase A: LN1 chunk -> K/V (+Q after chunk 0) -----
                    scopeA = nc.enter_named_scope("phA_ln1", False)
                    LCH = 512
                    # weight prefetches first: DMA runs under LN compute
                    w1bufs = 3 if fp8_kv else 2  # bf16 fallback shares w2m rotation
                    wq1_sb = wmain.tile([P, DT, D], wdt1, tag="w1a" if fp8_kv else "w2m",
                                        bufs=w1bufs)
                    nc.sync.dma_start(wq1_sb, wview(wq1T))
                    wk1_sb = wmain.tile([P, DT, D], wdt1, tag="w1a" if fp8_kv else "w2m",
                                        bufs=w1bufs)
                    nc.sync.dma_start(wk1_sb, wview(wk1T))
                    bk1 = bias_sb.get("bk1")
                    bv1 = bias_sb.get("bv1")

                    # ----- Phase B first: K2/V2 from context only — fills
                    # the PE while the LN1 stats chain + x DMAs warm up.
                    scopeB = nc.enter_named_scope("phB_qkv", False)
                    with tc.tile_pool(name="wb", bufs=1) as wpool, \
                         tc.tile_pool(name="projps2", bufs=2,
                                      space="PSUM") as pp:
                        ctx_sb = wpool.tile([P, CT, CN], BF16, tag="ctx",
                                            bufs=1)
                        for ct in range(CT):
                            nc.sync.dma_start(ctx_sb[:, ct, :], ctxT_v[ct])
                        wk2_sb = wpool.tile([P, CT, D], BF16, tag="w15",
                                            bufs=2)
                        nc.sync.dma_start(wk2_sb, wview(wk2T))
                        for oc in range(DT):
                            k_ps = pp.tile([P, CN], F32, tag="ppsm", bufs=2)
                            for it in range(CT):
                                nc.tensor.matmul(
                                    k_ps, wk2_sb[:, it, oc * P:(oc + 1) * P],
                                    ctx_sb[:, it, :],
                                    start=(it == 0), stop=(it == CT - 1))
                            nc.scalar.copy(out=K2_sb[:, oc, :], in_=k_ps)
                        wv2_sb = wpool.tile([P, CT, D], BF16, tag="w15",
                                            bufs=2)
                        nc.sync.dma_start(wv2_sb, wview(wv2T))
                        for hc in range(2):
                            v_ps = pp.tile([CN, 512], F32, tag="ppsm", bufs=2)
                            for it in range(CT):
                                nc.tensor.matmul(
                                    v_ps, ctx_sb[:, it, :],
                                    wv2_sb[:, it, hc * 512:(hc + 1) * 512],
                                    start=(it == 0), stop=(it == CT - 1))
                            nc.scalar.copy(
                                out=V2_sb[0:CN, hc * 8:(hc + 1) * 8, :],
                                in_=v_ps.rearrange("p (h d) -> p h d", d=64))
                    nc.leave_named_scope("phB_qkv", scopeB[0], False)

                    with tc.tile_pool(name="lnps", bufs=2, space="PSUM") as lnp, \
                         tc.tile_pool(name="lnsb", bufs=2) as lnsb:
                        nc.vector.memset(V_sb[:, :, :, 64:65], 1.0)
                        with tc.tile_pool(name="projps", bufs=2,
                                          space="PSUM") as pp:
                            wv1_sb = wmain.tile([P, DT, D], wdt1, tag="w1a" if fp8_kv else "w2m",
                                                bufs=w1bufs)
                            nc.sync.dma_start(wv1_sb, wview(wv1T))

                            def kv_for_chunk(tci):
                                if tci == 0:
                                    # Q for own tokens (= cols 0:TO, rotated)
                                    for oc in range(DT):
                                        q_full = pp.tile([P, 512], F32,
                                                         tag="pp512", bufs=2)
                                        q_ps = q_full[:, 0:TO]
                                        for it in range(DT):
                                            nc.tensor.matmul(
                                                q_ps,
                                                wq1_sb[:, it,
                                                       oc * P:(oc + 1) * P],
                                                ln1T[:, it, 0:TO],
                                                start=(it == 0),
                                                stop=(it == DT - 1
                                                      and "bq1" not in bias_sb))
                                        if "bq1" in bias_sb:
                                            nc.tensor.matmul(
                                                q_ps,
                                                bias_sb["bq1"][:,
                                                               oc * P:(oc + 1) * P],
                                                ones_b[:, :TO], start=False,
                                                stop=True)
                                        if fp8_kv:
                                            nc.scalar.mul(out=Q_sb[:, oc, :],
                                                          in_=q_ps, mul=dsq)
                                        else:
                                            nc.scalar.copy(out=Q_sb[:, oc, :],
                                                           in_=q_ps)
                                cols = slice(tci * 512, (tci + 1) * 512)
                                for oc in range(DT):
                                    k_ps = pp.tile([P, 512], F32, tag="pp512",
                                                   bufs=2)
                                    if fp8_kv:
                                        for itp in range(DT // 2):
                                            nc.tensor.matmul(
                                                k_ps,
                                                wk1_sb[:, 2 * itp:2 * itp + 2,
                                                       oc * P:(oc + 1) * P],
                                                ln1T[:, 2 * itp:2 * itp + 2,
                                                     cols],
                                                perf_mode=DRow,
                                                start=(itp == 0),
                                                stop=(itp == DT // 2 - 1))
                                        nc.vector.tensor_scalar_mul(
                                            out=K_sb[:, oc, cols], in0=k_ps,
                                            scalar1=dsk)
                                    else:
                                        for it in range(DT):
                                            nc.tensor.matmul(
                                                k_ps,
                                                wk1_sb[:, it,
                                                       oc * P:(oc + 1) * P],
                                                ln1T[:, it, cols],
                                                start=(it == 0),
                                                stop=(it == DT - 1
                                                      and bk1 is None))
                                        if bk1 is not None:
                                            nc.tensor.matmul(
                                                k_ps,
                                                bk1[:, oc * P:(oc + 1) * P],
                                                ones_b, start=False, stop=True)
                                        nc.vector.tensor_copy(
                                            out=K_sb[:, oc, cols], in_=k_ps)
                                for kt in range(tci * 4, tci * 4 + 4):
                                    for hc in range(2):
                                        v_ps = pp.tile([P, 512], F32,
                                                       tag="pp512", bufs=2)
                                        if fp8_kv:
                                            for itp in range(DT // 2):
                                                nc.tensor.matmul(
                                                    v_ps,
                                                    ln1T[:, 2 * itp:2 * itp + 2,
                                                         kt * P:(kt + 1) * P],
                                                    wv1_sb[:,
                                                           2 * itp:2 * itp + 2,
                                                           hc * 512:(hc + 1) * 512],
                                                    perf_mode=DRow,
                                                    start=(itp == 0),
                                                    stop=(itp == DT // 2 - 1))
                                        else:
                                            for it in range(DT):
                                                nc.tensor.matmul(
                                                    v_ps,
                                                    ln1T[:, it,
                                                         kt * P:(kt + 1) * P],
                                                    wv1_sb[:, it,
                                                           hc * 512:(hc + 1) * 512],
                                                    start=(it == 0),
                                                    stop=(it == DT - 1
                                                          and bv1 is None))
                                            if bv1 is not None:
                                                nc.tensor.matmul(
                                                    v_ps, ones_b[:, :P],
                                                    bv1[:,
                                                        hc * 512:(hc + 1) * 512],
                                                    start=False, stop=True)
                                        vsc = dsv if fp8_kv else AVS
                                        vdst = V_sb[:, kt,
                                                    hc * 8:(hc + 1) * 8, 0:64]
                                        vsrc = v_ps.rearrange(
                                            "p (h d) -> p h d", d=64)
                                        if hc:
                                            nc.vector.tensor_scalar_mul(
                                                out=vdst, in0=vsrc,
                                                scalar1=vsc)
                                        else:
                                            nc.scalar.mul(out=vdst, in_=vsrc,
                                                          mul=vsc)

                            def load_x(dt, tci, _c={}):
                                if (dt, tci) not in _c:
                                    t = lnsb.tile([P, LCH], F32R, tag="xt",
                                                  bufs=9)
                                    nc.sync.dma_start(
                                        t,
                                        xT_v[dt, :, tci * LCH:(tci + 1) * LCH])
                                    _c[(dt, tci)] = t
                                return _c[(dt, tci)]

                            _ln_feature_major(
                                nc, lnp, lnsb, consts, load_x,
                                lambda dt, tci: ln1T[:, dt,
                                                     tci * LCH:(tci + 1) * LCH],
                                DT, N, LCH, post_cb=kv_for_chunk,
                                bscale_row=(cl_row if fp8_kv else None))
                    nc.leave_named_scope("phA_ln1", scopeA[0], False)

                # ----- Phase C: self-attention heads -----
                scopeC = nc.enter_named_scope("phC_attn", False)
                # residual stream loads (first needed by phD)
                for dt in range(DT):
                    nc.sync.dma_start(x_ownT[:, dt, :], xT_v[dt, :, 0:TO])
                # prefetch next-phase weights under the attention stream
                wo1_sb = wmain.tile([P, DT, D], BF16, tag="w2m", bufs=2)
                nc.sync.dma_start(wo1_sb, wview(wo1T))
                wq2_sb = wmain.tile([P, DT, D], BF16, tag="w2m", bufs=2)
                nc.sync.dma_start(wq2_sb, wview(wq2T))
                with tc.tile_pool(name="aps", bufs=1, space="PSUM") as apsum, \
                     tc.tile_pool(name="asb", bufs=1) as asb:
                    for h in range(H):
                        j, r0 = h >> 1, (h & 1) * 64
                        o_ps = apsum.tile([65, TO], F32, tag="o_ps", bufs=2)
                        e_tiles = []

                        def av_block(g, o_ps=o_ps, h=h, e_tiles=e_tiles):
                            for i2 in range(2):
                                kt = g * 4 + 2 * i2
                                nc.tensor.matmul(
                                    o_ps, V_sb[:, kt:kt + 2, h, :],
                                    e_tiles[g][:, 2 * i2:2 * i2 + 2, :],
                                    perf_mode=DRow,
                                    start=(kt == 0), stop=(kt == NKT - 2))

                        for g in range(4):
                            # 4 key-tiles share one PSUM tile; one exp each
                            s_all = apsum.tile([P, 4, TO], F32, tag="s_all",
                                               bufs=2)
                            for q in range(4):
                                kt = g * 4 + q
                                # start=True on the first matmul touching
                                # each 2KB PSUM bank (pending-zero is
                                # bank-granular); never re-start a bank.
                                nc.tensor.matmul(
                                    s_all[:, q, :],
                                    K_sb[r0:r0 + 64, j, kt * P:(kt + 1) * P],
                                    Q_sb[r0:r0 + 64, j, :],
                                    start=(q % 2 == 0), stop=True,
                                    skip_group_check=(q > 0))
                            e_all = asb.tile([P, 4, TO], FP8, tag="e_all",
                                             bufs=3)
                            nc.scalar.activation(e_all, s_all, AF.Exp,
                                                 scale=SCALE)
                            e_tiles.append(e_all)
                            # AV for the previous tile group: its exp ran
                            # while this group's scores streamed, so the PE
                            # FIFO never parks on an exp wait.
                            if g >= 1:
                                av_block(g - 1)
                        av_block(3)
                        r_sb = asb.tile([1, TO], F32R, tag="r_sb", bufs=4)
                        with nc.allow_low_precision("f32r == f32 bits"):
                            nc.vector.reciprocal(r_sb, o_ps[64:65, :])
                        r_ps = apsum.tile([64, TO], F32, tag="r_ps", bufs=2)
                        nc.tensor.matmul(r_ps, invav_row[:, :64], r_sb,
                                         start=True, stop=True)
                        r_bc = asb.tile([64, TO], F32, tag="r_bc", bufs=3)
                        nc.vector.tensor_copy(out=r_bc, in_=r_ps)
                        nc.vector.tensor_tensor(out=O_sb[r0:r0 + 64, j, :],
                                                in0=o_ps[0:64, :],
                                                in1=r_bc, op=OP.mult)
                nc.leave_named_scope("phC_attn", scopeC[0], False)

                # ----- Phase D: attn1 out-proj + residual -----
                scopeD = nc.enter_named_scope("phD_oproj", False)
                with tc.tile_pool(name="dps", bufs=3, space="PSUM") as pp:
                    def add_residual(oc, ps):
                        nc.vector.tensor_tensor(
                            out=x_ownT[:, oc, :],
                            in0=x_ownT[:, oc, :].bitcast(F32),
                            in1=ps, op=OP.add)

                    proj_feature_major(pp, wo1_sb, O_sb, add_residual, DT, TO,
                                       bias=bias_sb.get("bo1"))
                nc.leave_named_scope("phD_oproj", scopeD[0], False)

            # ========== attn2 scope: phase E ==========
            scopeE = nc.enter_named_scope("phE_xattn", False)
            with tc.tile_pool(name="ce", bufs=1) as ce:
                ln2T = ce.tile([P, DT, TO], BF16)
                Q2_sb = ce.tile([P, DT, TO], BF16)
                O2_sb = ce.tile([P, DT, TO], BF16)

                with tc.tile_pool(name="lnps2", bufs=2, space="PSUM") as lnp, \
                     tc.tile_pool(name="lnsb2", bufs=2) as lnsb:
                    _ln_feature_major(
                        nc, lnp, lnsb, consts,
                        lambda dt, tci: x_ownT[:, dt, :],
                        lambda dt, tci: ln2T[:, dt, :],
                        DT, TO, TO, alt_mult=True)

                wo2_sb = wmain.tile([P, DT, D], BF16, tag="w2m", bufs=2)
                with tc.tile_pool(name="eps_", bufs=2, space="PSUM") as pp:
                    proj_feature_major(
                        pp, wq2_sb, ln2T,
                        lambda oc, ps: nc.scalar.copy(out=Q2_sb[:, oc, :],
                                                      in_=ps),
                        DT, TO, bias=bias_sb.get("bq2"))
                    nc.sync.dma_start(wo2_sb, wview(wo2T))

                with tc.tile_pool(name="aps2", bufs=1, space="PSUM") as apsum, \
                     tc.tile_pool(name="asb2", bufs=1) as asb:
                    for grp in range(4):
                        den_ps = apsum.tile([P, 2, 4], F32, tag="den", bufs=1)
                        o_all = apsum.tile([64, 4, TO], F32, tag="o_all",
                                           bufs=1)
                        e_list = []
                        s4 = apsum.tile([CN, 4, TO], F32, tag="s_ps", bufs=1)
                        for h8 in range(4):
                            h = grp * 4 + h8
                            j, r0 = h >> 1, (h & 1) * 64
                            nc.tensor.matmul(
                                s4[:, h8, :], K2_sb[r0:r0 + 64, j, :],
                                Q2_sb[r0:r0 + 64, j, :],
                                start=(h8 % 2 == 0), stop=True,
                                skip_group_check=(h8 > 0))
                            e_t = asb.tile([CN, TO], BF16, tag="e_t", bufs=8)
                            nc.scalar.activation(e_t, s4[:, h8, :], AF.Exp,
                                                 scale=SCALE)
                            e_list.append(e_t)
                        for h8 in range(4):
                            h = grp * 4 + h8
                            e_t = e_list[h8]
                            # [64, 4, 256] f32 = 2 banks: h8 0/1 in bank 0,
                            # h8 2/3 in bank 1 -> start on h8 0 and 2 only.
                            nc.tensor.matmul(o_all[:, h8, :], V2_sb[0:CN, h, :],
                                             e_t, start=(h8 % 2 == 0),
                                             stop=True,
                                             skip_group_check=(h8 > 0))
                            # denominators: den[q, c, h8] = sum_k e[k, q]
                            for c in range(2):
                                nc.tensor.matmul(
                                    den_ps[:, c, h8:h8 + 1],
                                    e_t[:, c * P:(c + 1) * P],
                                    ones_colb[0:CN, :],
                                    start=(h8 == 0 and c == 0), stop=True,
                                    skip_group_check=(h8 > 0 or c > 0))
                        rf = asb.tile([P, 2, 4], BF16, tag="rf", bufs=2)
                        with nc.allow_low_precision("softmax denom to bf16"):
                            nc.vector.reciprocal(rf, den_ps)
                        rT_ps = apsum.tile([8, P], BF16, tag="rT", bufs=1)
                        nc.tensor.transpose(rT_ps, rf.rearrange("p c h -> p (c h)"),
                                            ident_sb)
                        rT_sb = asb.tile([8, P], BF16, tag="rTs", bufs=2)
                        nc.vector.tensor_copy(out=rT_sb, in_=rT_ps)
                        for h8 in range(4):
                            h = grp * 4 + h8
                            j, r0 = h >> 1, (h & 1) * 64
                            r_ps = apsum.tile([64, 2, P], F32, tag="r_ps",
                                              bufs=1)
                            for c in range(2):
                                idx = c * 4 + h8
                                nc.tensor.matmul(
                                    r_ps[:, c, :],
                                    sel_sb[0:8, idx * 64:(idx + 1) * 64],
                                    rT_sb, start=(c == 0), stop=True,
                                    skip_group_check=(c > 0))
                            r_bc = asb.tile([64, TO], F32, tag="r_bc", bufs=3)
                            nc.scalar.copy(
                                out=r_bc.rearrange("p (c q) -> p c q", c=2),
                                in_=r_ps)
                            nc.vector.tensor_tensor(
                                out=O2_sb[r0:r0 + 64, j, :],
                                in0=o_all[:, h8, :], in1=r_bc, op=OP.mult)

                with tc.tile_pool(name="eps2", bufs=3, space="PSUM") as pp:
                    def add_residual2(oc, ps):
                        nc.vector.tensor_tensor(
                            out=x_ownT[:, oc, :],
                            in0=x_ownT[:, oc, :].bitcast(F32),
                            in1=ps, op=OP.add)

                    proj_feature_major(pp, wo2_sb, O2_sb, add_residual2, DT, TO,
                                       bias=bias_sb.get("bo2"))
            nc.leave_named_scope("phE_xattn", scopeE[0], False)

            # ========== FFN scope: phase F ==========
            scopeF = nc.enter_named_scope("phF_ffn", False)
            with tc.tile_pool(name="cf", bufs=1) as cf:
                ln3T = cf.tile([P, DT, TO], BF16)
                Hbuf = cf.tile([P, FT, TO], BF16)

                wgT_v = wview(wgT)
                wfT_v = wfT.rearrange("(f p) o -> f p o", p=P)
                bgeg = bias_sb.get("bgeg")
                bff = bias_sb.get("bff")
                # interleave GEGLU groups with ffout blocks: per fg (8 f-tiles)
                # compute Hbuf then immediately contract into the output.
                with tc.tile_pool(name="wg", bufs=1) as wgpool, \
                     tc.tile_pool(name="wfp", bufs=1) as wfpool:

                    def load_wg(g):
                        wg_h = wgpool.tile([P, DT, 512], BF16, tag="wgh",
                                           bufs=2)
                        nc.sync.dma_start(
                            wg_h, wgT_v[:, :, g * 512:(g + 1) * 512])
                        wg_g = wgpool.tile([P, DT, 512], BF16, tag="wgg",
                                           bufs=2)
                        nc.sync.dma_start(
                            wg_g,
                            wgT_v[:, :, FF + g * 512:FF + (g + 1) * 512])
                        return wg_h, wg_g

                    def load_wf(fg):
                        tiles = []
                        for f8 in range(8):
                            wt = wfpool.tile([P, D], BF16, tag="wft", bufs=16)
                            nc.sync.dma_start(wt, wfT_v[fg * 8 + f8])
                            tiles.append(wt)
                        return tiles

                    # first weight groups prefetch under the LN3 compute so
                    # the PE has work the moment LN3 lands
                    wg0 = load_wg(0)
                    wf0 = load_wf(0)
                    with tc.tile_pool(name="lnps3", bufs=2,
                                      space="PSUM") as lnp, \
                         tc.tile_pool(name="lnsb3", bufs=2) as lnsb:
                        _ln_feature_major(
                            nc, lnp, lnsb, consts,
                            lambda dt, tci: x_ownT[:, dt, :],
                            lambda dt, tci: ln3T[:, dt, :],
                            DT, TO, TO, alt_mult=True)

                    ffn_pools = tc.tile_pool(name="gps", bufs=1, space="PSUM")
                    gpsum = ffn_pools.__enter__()
                    yp_pools = tc.tile_pool(name="yps", bufs=2, space="PSUM")
                    yp_ = yp_pools.__enter__()
                    gsb_pools = tc.tile_pool(name="gsb", bufs=3)
                    gsb = gsb_pools.__enter__()
                    for fg in range(4):
                        wf_tiles = wf0 if fg == 0 else load_wf(fg)
                        for g2 in range(2):
                            g = fg * 2 + g2
                            wg_h, wg_g = wg0 if g == 0 else load_wg(g)
                            for fi in range(4):
                                f = g * 4 + fi
                                h_ps = gpsum.tile([P, TO], F32, tag="h_ps",
                                                  bufs=2)
                                for it in range(DT):
                                    nc.tensor.matmul(
                                        h_ps, wg_h[:, it, fi * P:(fi + 1) * P],
                                        ln3T[:, it, :],
                                        start=(it == 0),
                                        stop=(it == DT - 1 and bgeg is None))
                                if bgeg is not None:
                                    nc.tensor.matmul(
                                        h_ps, bgeg[:, f * P:(f + 1) * P],
                                        ones_b[:, :TO], start=False, stop=True)
                                g_ps = gpsum.tile([P, TO], F32, tag="g_ps",
                                                  bufs=2)
                                for it in range(DT):
                                    nc.tensor.matmul(
                                        g_ps, wg_g[:, it, fi * P:(fi + 1) * P],
                                        ln3T[:, it, :],
                                        start=(it == 0),
                                        stop=(it == DT - 1 and bgeg is None))
                                if bgeg is not None:
                                    nc.tensor.matmul(
                                        g_ps,
                                        bgeg[:, FF + f * P:FF + (f + 1) * P],
                                        ones_b[:, :TO], start=False, stop=True)
                                gel = gsb.tile([P, TO], F32, tag="gel", bufs=3)
                                nc.scalar.activation(gel, g_ps, AF.Gelu)
                                nc.vector.tensor_tensor(out=Hbuf[:, f, :],
                                                        in0=h_ps, in1=gel,
                                                        op=OP.mult)
                        # ffout for this fg block (two-level accumulation;
                        # spills add into x_ownT)
                        for oc in range(DT):
                            i_ps = yp_.tile([P, TO], F32, tag="i_ps")
                            add_bias = bff is not None and fg == 3
                            for f8 in range(8):
                                nc.tensor.matmul(
                                    i_ps, wf_tiles[f8][:, oc * P:(oc + 1) * P],
                                    Hbuf[:, fg * 8 + f8, :],
                                    start=(f8 == 0),
                                    stop=(f8 == 7 and not add_bias))
                            if add_bias:
                                nc.tensor.matmul(
                                    i_ps, bff[:, oc * P:(oc + 1) * P],
                                    ones_b[:, :TO], start=False, stop=True)
                            nc.vector.tensor_tensor(
                                out=x_ownT[:, oc, :],
                                in0=x_ownT[:, oc, :].bitcast(F32),
                                in1=i_ps, op=OP.add)
                            if fg == 3:
                                nc.sync.dma_start(yT_v[:, oc, :],
                                                  x_ownT[:, oc, :])
                    gsb_pools.__exit__(None, None, None)
                    yp_pools.__exit__(None, None, None)
                    ffn_pools.__exit__(None, None, None)
            nc.leave_named_scope("phF_ffn", scopeF[0], False)

    nc.finalize()
    return nc


_CACHE = {}


def kernel(**inputs):
    def f32c(a):
        return np.ascontiguousarray(np.asarray(a, dtype=np.float32))

    def bfT(w):
        """W [out,in] (optionally gain-folded) -> bf16 W.T contiguous."""
        return np.ascontiguousarray(w.T).astype(ml_dtypes.bfloat16)

    x = f32c(inputs["hidden_states"])[0]          # [N, D]
    ctx = f32c(inputs["context"])[0]              # [CN, CD]
    g1 = f32c(inputs["ln1_g"]); b1 = f32c(inputs["ln1_b"])
    g2 = f32c(inputs["ln2_g"]); b2 = f32c(inputs["ln2_b"])
    g3 = f32c(inputs["ln3_g"]); b3 = f32c(inputs["ln3_b"])
    wq1 = f32c(inputs["wq1"]); wk1 = f32c(inputs["wk1"]); wv1 = f32c(inputs["wv1"])
    wo1 = f32c(inputs["wo1"]); bo1 = f32c(inputs["bo1"])
    wq2 = f32c(inputs["wq2"]); wk2 = f32c(inputs["wk2"]); wv2 = f32c(inputs["wv2"])
    wo2 = f32c(inputs["wo2"]); bo2 = f32c(inputs["bo2"])
    wg = f32c(inputs["w_geglu"]); bg = f32c(inputs["b_geglu"])
    wf = f32c(inputs["w_ffout"]); bf = f32c(inputs["b_ffout"])

    bq1 = wq1 @ b1; bk1 = wk1 @ b1; bv1 = wv1 @ b1
    bq2 = wq2 @ b2
    bgeg = bg + wg @ b3
    flags = (bool(np.any(bq1) or np.any(bk1) or np.any(bv1)), bool(np.any(bo1)),
             bool(np.any(bq2)), bool(np.any(bo2)), bool(np.any(bgeg)),
             bool(np.any(bf)))

    def pow2scale(w):
        a = float(np.abs(w).max())
        if a <= 0:
            return 1.0
        return float(2.0 ** int(np.floor(np.log2(224.0 / a))))

    wq1g = wq1 * g1[None, :]
    wk1g = wk1 * g1[None, :]
    wv1g = wv1 * g1[None, :]
    fp8_kv = not flags[0]
    qexps = (pow2scale(wq1g), pow2scale(wk1g),
             pow2scale(wv1g)) if fp8_kv else None

    key = (flags, qexps)
    if key not in _CACHE:
        _CACHE[key] = build(flags, qexps)
    nc = _CACHE[key]

    xT = np.ascontiguousarray(x.T)                # [D, N]
    bf16 = ml_dtypes.bfloat16
    fp8 = ml_dtypes.float8_e4m3

    def f8T(w, s):
        return np.ascontiguousarray(w.T * s).astype(fp8)

    selm = np.zeros((16, 1024), np.float32)
    for r in range(16):
        selm[r, r * 64:(r + 1) * 64] = 1.0
    shared = {
        "ctxT": np.ascontiguousarray(ctx.T).astype(bf16),
        "wq1T": f8T(wq1g, qexps[0]) if fp8_kv else bfT(wq1g),
        "wk1T": f8T(wk1g, qexps[1]) if fp8_kv else bfT(wk1g),
        "wv1T": f8T(wv1g, qexps[2]) if fp8_kv else bfT(wv1g),
        "wo1T": bfT(wo1),
        "wq2T": bfT(wq2 * g2[None, :]),
        "wk2T": bfT(wk2),
        "wv2T": bfT(wv2),
        "wo2T": bfT(wo2),
        "wgT": bfT(wg * g3[None, :]),
        "wfT": bfT(wf),
        "onesc": np.ones((P, 1), np.float32),
        "onescb": np.ones((P, 1), bf16),
        "onesr": np.ones((1, P), np.float32),
        "invavr": np.full((1, P), 1.0 / AVS, np.float32),
        "onesb": np.ones((1, 512), bf16),
        "selm": selm.astype(bf16),
        "identb": np.eye(P, dtype=np.float32).astype(bf16),
    }
    if flags[0]:
        shared["bq1"] = bq1[None, :].astype(bf16)
        shared["bk1"] = bk1[None, :].astype(bf16)
        shared["bv1"] = bv1[None, :].astype(bf16)
    if flags[1]:
        shared["bo1"] = bo1[None, :].astype(bf16)
    if flags[2]:
        shared["bq2"] = bq2[None, :].astype(bf16)
    if flags[3]:
        shared["bo2"] = bo2[None, :].astype(bf16)
    if flags[4]:
        shared["bgeg"] = bgeg[None, :].astype(bf16)
    if flags[5]:
        shared["bff"] = bf[None, :].astype(bf16)

    in_maps = []
    for c in range(NCORES):
        m = dict(shared)
        # rotate so core c's own tokens occupy columns 0:TO
        m["xT"] = np.ascontiguousarray(np.roll(xT, -c * TO, axis=1))
        in_maps.append(m)

    res = run_bass_kernel_spmd(nc, in_maps, core_ids=list(range(NCORES)))
    yT = np.concatenate([r["yT"] for r in res.results], axis=1)  # [D, N]
    return np.ascontiguousarray(yT.T)[None].astype(np.float32)


# revision 76
# speedup vs baseline: 1.0100x; 1.0100x over previous
"""Trainium2 Bass kernel for nn_BasicTransformerBlock (self-attn + cross-attn + GEGLU).

Sharding: data-parallel over the 2048 tokens (256 per core, 8 cores, no
collectives). K/V for self-attention are computed replicated on every core.
Each core's xT input is rotated so its own 256 tokens are columns 0:256 —
softmax over keys is permutation-invariant, so K/V order doesn't matter,
and Q/LN-own can slice the full-sequence LN output directly.

On-chip layout is feature-major throughout ([feature(part), token(free)]).
Host pre-packs weights as bf16 W.T (C-contiguous [in, out]) and pre-transposes
x / context, so the device does zero transposes/casts and all DMAs are
contiguous. Weight/projection matmuls run in bf16 (fp32 PSUM accumulate);
LayerNorm statistics run in float32r off the fp32 residual stream; rstd is
exp(-0.5*ln(var+eps)) on ScalarE (same activation-table set as attention exp).

Softmax (attn1): scores keys-on-partitions ([keys, q]); exp batched 4 key-tiles
per activation; denominators from an appended ones-column in V (row 64 of the
AV accumulation); per-head reciprocal hides under the exp stream.
Softmax (attn2): denominators via e.T@ones matmuls into a [q,*] PSUM tile,
one batched reciprocal per 8-head group, PE-transpose + selector-broadcast
to get per-head [1,q] rows back.
"""

import numpy as np
import ml_dtypes

import concourse.bass as bass
import concourse.mybir as mybir
import concourse.tile as tile
from concourse import bacc
from concourse.bass_utils import run_bass_kernel_spmd

F32 = mybir.dt.float32
F32R = mybir.dt.float32r
BF16 = mybir.dt.bfloat16
FP8 = mybir.dt.float8e4
DRow = mybir.MatmulPerfMode.DoubleRow
AF = mybir.ActivationFunctionType
OP = mybir.AluOpType

AE = 8.0      # (unused) attn1 exp output fp8 scale
AVS = 32.0    # V fp8 scale (folded out via the reciprocal broadcast row)
CL = 32.0     # LN1 output fp8 scale (folded into the rstd broadcast row)

P = 128
N, D = 2048, 1024
H, DH = 16, 64
CN, CD = 77, 768
FF = 4096
EPS = 1e-5
SCALE = DH ** -0.5
NCORES = 8
TO = N // NCORES          # 256 tokens owned per core
DT = D // P               # 8 feature tiles
CT = CD // P              # 6 context-feature tiles
NKT = N // P              # 16 key tiles
FT = FF // P              # 32 ffn-inner tiles


def _ln_feature_major(nc, lnp, sbp, consts, src_of, dst_of, n_dt, tn, chunk,
                      post_cb=None, bscale_row=None, alt_mult=False):
    """Un-affine LayerNorm over feature-major f32r data.

    bscale_row: optional [1,P] const row used as the rstd broadcast lhsT;
    a row of value c makes the output c*(x-mu)*rstd (fp8 pre-scale).
    """
    ones_col, ones_row, eps_t = consts
    inv_d = 1.0 / (n_dt * P)
    for tci in range(tn // chunk):
        srcs = [src_of(dt, tci) for dt in range(n_dt)]   # f32r tiles
        sum_ps = lnp.tile([1, chunk], F32, tag="ln_sum", bufs=2)
        for dt in range(n_dt):
            nc.tensor.matmul(sum_ps, ones_col, srcs[dt],
                             start=(dt == 0), stop=(dt == n_dt - 1))
        sumsq_ps = lnp.tile([1, chunk], F32, tag="ln_sumsq", bufs=2)
        for dt in range(n_dt):
            sq_t = sbp.tile([P, chunk], F32R, tag="ln_sq", bufs=3)
            if dt % 2:
                nc.scalar.activation(sq_t, srcs[dt].bitcast(F32), AF.Square)
            else:
                nc.gpsimd.tensor_tensor(out=sq_t, in0=srcs[dt].bitcast(F32),
                                        in1=srcs[dt].bitcast(F32), op=OP.mult)
            nc.tensor.matmul(sumsq_ps, ones_col, sq_t,
                             start=(dt == 0), stop=(dt == n_dt - 1))
        mu_row = sbp.tile([1, chunk], F32R, tag="ln_mu", bufs=2)
        nc.scalar.mul(out=mu_row, in_=sum_ps, mul=inv_d)
        var_row = sbp.tile([1, chunk], F32, tag="ln_var", bufs=2)
        nc.scalar.mul(out=var_row, in_=sumsq_ps, mul=inv_d)
        musq = sbp.tile([1, chunk], F32, tag="ln_musq", bufs=2)
        nc.vector.tensor_mul(out=musq, in0=mu_row.bitcast(F32),
                             in1=mu_row.bitcast(F32))
        nc.vector.tensor_tensor(out=var_row, in0=var_row, in1=musq,
                                op=OP.subtract)
        # rstd = (var+eps)^-0.5 via exp(-0.5*ln(var+eps)): stays in the
        # natural_log/exp activation-table family, no DVE reciprocal.
        lnv_row = sbp.tile([1, chunk], F32, tag="ln_lnv", bufs=2)
        nc.scalar.activation(lnv_row, var_row, AF.Ln, bias=eps_t)
        rstd_row = sbp.tile([1, chunk], F32R, tag="ln_rstd", bufs=2)
        nc.scalar.activation(rstd_row, lnv_row, AF.Exp, scale=-0.5)
        mu_b = lnp.tile([P, chunk], F32, tag="ln_mub", bufs=1)
        nc.tensor.matmul(mu_b, ones_row, mu_row, start=True, stop=True)
        rstd_b = lnp.tile([P, chunk], F32, tag="ln_rstdb", bufs=1)
        nc.tensor.matmul(rstd_b,
                         ones_row if bscale_row is None else bscale_row,
                         rstd_row, start=True, stop=True)
        mu_s = sbp.tile([P, chunk], F32, tag="ln_mus", bufs=2)
        nc.scalar.copy(out=mu_s, in_=mu_b)
        rstd_s = None
        if alt_mult:
            # SBUF copy so the Pool engine (no PSUM port) can do half
            # the normalize multiplies.
            rstd_s = sbp.tile([P, chunk], F32, tag="ln_rstds", bufs=2)
            nc.scalar.copy(out=rstd_s, in_=rstd_b)
        for dt in range(n_dt):
            tmp = sbp.tile([P, chunk], F32, tag="ln_tmp", bufs=3)
            eng = nc.gpsimd if dt % 2 else nc.vector
            src_in = mu_s if dt % 2 else mu_b
            eng.tensor_tensor(out=tmp, in0=srcs[dt].bitcast(F32),
                              in1=src_in, op=OP.subtract)
            if alt_mult and dt % 2:
                nc.gpsimd.tensor_tensor(out=dst_of(dt, tci), in0=tmp,
                                        in1=rstd_s, op=OP.mult)
            else:
                nc.vector.tensor_tensor(out=dst_of(dt, tci), in0=tmp,
                                        in1=rstd_b, op=OP.mult)
        if post_cb is not None:
            post_cb(tci)


def build(flags, qexps=None):
    has_qkv1b, has_bo1, has_q2b, has_bo2, has_gegb, has_ffb = flags
    fp8_kv = qexps is not None and not has_qkv1b
    nc = bacc.Bacc()

    wdt1 = FP8 if fp8_kv else BF16
    xT = nc.dram_tensor("xT", [D, N], F32R, kind="ExternalInput")
    ctxT = nc.dram_tensor("ctxT", [CD, CN], BF16, kind="ExternalInput")
    wq1T = nc.dram_tensor("wq1T", [D, D], wdt1, kind="ExternalInput")
    wk1T = nc.dram_tensor("wk1T", [D, D], wdt1, kind="ExternalInput")
    wv1T = nc.dram_tensor("wv1T", [D, D], wdt1, kind="ExternalInput")
    wo1T = nc.dram_tensor("wo1T", [D, D], BF16, kind="ExternalInput")
    wq2T = nc.dram_tensor("wq2T", [D, D], BF16, kind="ExternalInput")
    wk2T = nc.dram_tensor("wk2T", [CD, D], BF16, kind="ExternalInput")
    wv2T = nc.dram_tensor("wv2T", [CD, D], BF16, kind="ExternalInput")
    wo2T = nc.dram_tensor("wo2T", [D, D], BF16, kind="ExternalInput")
    wgT = nc.dram_tensor("wgT", [D, 2 * FF], BF16, kind="ExternalInput")
    wfT = nc.dram_tensor("wfT", [FF, D], BF16, kind="ExternalInput")
    onesc = nc.dram_tensor("onesc", [P, 1], F32R, kind="ExternalInput")
    onescb = nc.dram_tensor("onescb", [P, 1], BF16, kind="ExternalInput")
    onesr = nc.dram_tensor("onesr", [1, P], F32R, kind="ExternalInput")
    invavr = nc.dram_tensor("invavr", [1, P], F32R, kind="ExternalInput")
    onesb = nc.dram_tensor("onesb", [1, 512], BF16, kind="ExternalInput")
    selm = nc.dram_tensor("selm", [16, 1024], BF16, kind="ExternalInput")
    identb = nc.dram_tensor("identb", [P, P], BF16, kind="ExternalInput")
    bias_rows = {}
    if has_qkv1b:
        for nm in ("bq1", "bk1", "bv1"):
            bias_rows[nm] = nc.dram_tensor(nm, [1, D], BF16, kind="ExternalInput")
    if has_bo1:
        bias_rows["bo1"] = nc.dram_tensor("bo1", [1, D], BF16, kind="ExternalInput")
    if has_q2b:
        bias_rows["bq2"] = nc.dram_tensor("bq2", [1, D], BF16, kind="ExternalInput")
    if has_bo2:
        bias_rows["bo2"] = nc.dram_tensor("bo2", [1, D], BF16, kind="ExternalInput")
    if has_gegb:
        bias_rows["bgeg"] = nc.dram_tensor("bgeg", [1, 2 * FF], BF16,
                                           kind="ExternalInput")
    if has_ffb:
        bias_rows["bff"] = nc.dram_tensor("bff", [1, D], BF16, kind="ExternalInput")
    yT = nc.dram_tensor("yT", [D, TO], F32R, kind="ExternalOutput")

    xT_v = xT.rearrange("(dt p) t -> dt p t", p=P)
    ctxT_v = ctxT.rearrange("(ct p) t -> ct p t", p=P)
    yT_v = yT.rearrange("(dt p) t -> p dt t", p=P)

    def wview(w):
        return w.rearrange("(it p) o -> p it o", p=P)

    with tile.TileContext(nc) as tc:
        with tc.tile_pool(name="consts", bufs=1) as cpool, \
             tc.tile_pool(name="pers", bufs=1) as pers, \
             tc.tile_pool(name="wmain", bufs=1) as wmain:

            ones_col = cpool.tile([P, 1], F32R)
            nc.sync.dma_start(ones_col, onesc[:])
            ones_colb = cpool.tile([P, 1], BF16)
            nc.sync.dma_start(ones_colb, onescb[:])
            ones_row = cpool.tile([1, P], F32R)
            nc.sync.dma_start(ones_row, onesr[:])
            ones_b = cpool.tile([1, 512], BF16)
            nc.sync.dma_start(ones_b, onesb[:])
            invav_row = cpool.tile([1, P], F32R)
            nc.sync.dma_start(invav_row, invavr[:])
            cl_f32 = cpool.tile([1, P], F32)
            nc.vector.memset(cl_f32, CL)
            cl_row = cl_f32.bitcast(F32R)
            sel_sb = cpool.tile([16, 1024], BF16)
            nc.sync.dma_start(sel_sb, selm[:])
            ident_sb = cpool.tile([P, P], BF16)
            nc.sync.dma_start(ident_sb, identb[:])
            eps_t = cpool.tile([1, 1], F32)
            nc.vector.memset(eps_t, EPS)
            consts = (ones_col, ones_row, eps_t)

            bias_sb = {}
            for nm, t in bias_rows.items():
                bt = cpool.tile([1, t.shape[1]], BF16, tag=f"bias_{nm}")
                nc.sync.dma_start(bt, t[:])
                bias_sb[nm] = bt

            def proj_feature_major(pp, w_sb, act, out_cb, n_in, n_tok,
                                   bias=None, tag="pp256"):
                """out[oc] = sum_it w.T @ act; out_cb(oc, psum)."""
                for oc in range(DT):
                    ps = pp.tile([P, n_tok], F32, tag=tag, bufs=2)
                    for it in range(n_in):
                        nc.tensor.matmul(ps, w_sb[:, it, oc * P:(oc + 1) * P],
                                         act[:, it, :],
                                         start=(it == 0),
                                         stop=(it == n_in - 1 and bias is None))
                    if bias is not None:
                        nc.tensor.matmul(ps, bias[:, oc * P:(oc + 1) * P],
                                         ones_b[:, :n_tok], start=False,
                                         stop=True)
                    out_cb(oc, ps)

            x_ownT = pers.tile([P, DT, TO], F32R)      # residual stream (own)
            for dt in range(DT):
                nc.sync.dma_start(x_ownT[:, dt, :], xT_v[dt, :, 0:TO])

            # cross-attn K2/V2 depend only on the context: computed early in
            # phase B so they overlap everything up to phase E.
            K2_sb = pers.tile([P, DT, CN], BF16)
            V2_sb = pers.tile([P, H, DH], BF16)

            # ========== attn1 scope: phases A-D ==========
            with tc.tile_pool(name="c1", bufs=1) as c1:
                O_sb = c1.tile([P, DT, TO], BF16)
                K_sb = c1.tile([P, DT, N], BF16)
                V_sb = c1.tile([P, NKT, H, 65], FP8)
                Q_sb = c1.tile([P, DT, TO], BF16)

                with tc.tile_pool(name="c2", bufs=1) as c2:
                    ln1T = c2.tile([P, DT, N], FP8 if fp8_kv else BF16)
                    if fp8_kv:
                        swq, swk, swv = qexps
                        dsq = 1.0 / (CL * swq)
                        dsk = 1.0 / (CL * swk)
                        dsv = AVS / (CL * swv)

                    # ----- Phase A: LN1 chunk -> K/V (+Q after chunk 0) -----
                    scopeA = nc.enter_named_scope("phA_ln1", False)
                    LCH = 512
                    # weight prefetches first: DMA runs under LN compute
                    w1bufs = 3 if fp8_kv else 2  # bf16 fallback shares w2m rotation
                    wq1_sb = wmain.tile([P, DT, D], wdt1, tag="w1a" if fp8_kv else "w2m",
                                        bufs=w1bufs)
                    nc.scalar.dma_start(wq1_sb, wview(wq1T))
                    wk1_sb = wmain.tile([P, DT, D], wdt1, tag="w1a" if fp8_kv else "w2m",
                                        bufs=w1bufs)
                    nc.scalar.dma_start(wk1_sb, wview(wk1T))
                    bk1 = bias_sb.get("bk1")
                    bv1 = bias_sb.get("bv1")
                    with tc.tile_pool(name="lnps", bufs=2, space="PSUM") as lnp, \
                         tc.tile_pool(name="lnsb", bufs=2) as lnsb:
                        nc.vector.memset(V_sb[:, :, :, 64:65], 1.0)
                        with tc.tile_pool(name="projps", bufs=2,
                                          space="PSUM") as pp:
                            wv1_sb = wmain.tile([P, DT, D], wdt1, tag="w1a" if fp8_kv else "w2m",
                                                bufs=w1bufs)
                            nc.scalar.dma_start(wv1_sb, wview(wv1T))

                            def kv_for_chunk(tci):
                                if tci == 0:
                                    # Q for own tokens (= cols 0:TO, rotated)
                                    for oc in range(DT):
                                        q_full = pp.tile([P, 512], F32,
                                                         tag="pp512", bufs=2)
                                        q_ps = q_full[:, 0:TO]
                                        for it in range(DT):
                                            nc.tensor.matmul(
                                                q_ps,
                                                wq1_sb[:, it,
                                                       oc * P:(oc + 1) * P],
                                                ln1T[:, it, 0:TO],
                                                start=(it == 0),
                                                stop=(it == DT - 1
                                                      and "bq1" not in bias_sb))
                                        if "bq1" in bias_sb:
                                            nc.tensor.matmul(
                                                q_ps,
                                                bias_sb["bq1"][:,
                                                               oc * P:(oc + 1) * P],
                                                ones_b[:, :TO], start=False,
                                                stop=True)
                                        if fp8_kv:
                                            nc.scalar.mul(out=Q_sb[:, oc, :],
                                                          in_=q_ps, mul=dsq)
                                        else:
                                            nc.scalar.copy(out=Q_sb[:, oc, :],
                                                           in_=q_ps)
                                cols = slice(tci * 512, (tci + 1) * 512)
                                for oc in range(DT):
                                    k_ps = pp.tile([P, 512], F32, tag="pp512",
                                                   bufs=2)
                                    if fp8_kv:
                                        for itp in range(DT // 2):
                                            nc.tensor.matmul(
                                                k_ps,
                                                wk1_sb[:, 2 * itp:2 * itp + 2,
                                                       oc * P:(oc + 1) * P],
                                                ln1T[:, 2 * itp:2 * itp + 2,
                                                     cols],
                                                perf_mode=DRow,
                                                start=(itp == 0),
                                                stop=(itp == DT // 2 - 1))
                                        nc.vector.tensor_scalar_mul(
                                            out=K_sb[:, oc, cols], in0=k_ps,
                                            scalar1=dsk)
                                    else:
                                        for it in range(DT):
                                            nc.tensor.matmul(
                                                k_ps,
                                                wk1_sb[:, it,
                                                       oc * P:(oc + 1) * P],
                                                ln1T[:, it, cols],
                                                start=(it == 0),
                                                stop=(it == DT - 1
                                                      and bk1 is None))
                                        if bk1 is not None:
                                            nc.tensor.matmul(
                                                k_ps,
                                                bk1[:, oc * P:(oc + 1) * P],
                                                ones_b, start=False, stop=True)
                                        nc.vector.tensor_copy(
                                            out=K_sb[:, oc, cols], in_=k_ps)
                                for kt in range(tci * 4, tci * 4 + 4):
                                    for hc in range(2):
                                        v_ps = pp.tile([P, 512], F32,
                                                       tag="pp512", bufs=2)
                                        if fp8_kv:
                                            for itp in range(DT // 2):
                                                nc.tensor.matmul(
                                                    v_ps,
                                                    ln1T[:, 2 * itp:2 * itp + 2,
                                                         kt * P:(kt + 1) * P],
                                                    wv1_sb[:,
                                                           2 * itp:2 * itp + 2,
                                                           hc * 512:(hc + 1) * 512],
                                                    perf_mode=DRow,
                                                    start=(itp == 0),
                                                    stop=(itp == DT // 2 - 1))
                                        else:
                                            for it in range(DT):
                                                nc.tensor.matmul(
                                                    v_ps,
                                                    ln1T[:, it,
                                                         kt * P:(kt + 1) * P],
                                                    wv1_sb[:, it,
                                                           hc * 512:(hc + 1) * 512],
                                                    start=(it == 0),
                                                    stop=(it == DT - 1
                                                          and bv1 is None))
                                            if bv1 is not None:
                                                nc.tensor.matmul(
                                                    v_ps, ones_b[:, :P],
                                                    bv1[:,
                                                        hc * 512:(hc + 1) * 512],
                                                    start=False, stop=True)
                                        vsc = dsv if fp8_kv else AVS
                                        vdst = V_sb[:, kt,
                                                    hc * 8:(hc + 1) * 8, 0:64]
                                        vsrc = v_ps.rearrange(
                                            "p (h d) -> p h d", d=64)
                                        if hc:
                                            nc.vector.tensor_scalar_mul(
                                                out=vdst, in0=vsrc,
                                                scalar1=vsc)
                                        else:
                                            nc.scalar.mul(out=vdst, in_=vsrc,
                                                          mul=vsc)

                            def load_x(dt, tci, _c={}):
                                if (dt, tci) not in _c:
                                    t = lnsb.tile([P, LCH], F32R, tag="xt",
                                                  bufs=9)
                                    nc.sync.dma_start(
                                        t,
                                        xT_v[dt, :, tci * LCH:(tci + 1) * LCH])
                                    _c[(dt, tci)] = t
                                return _c[(dt, tci)]

                            _ln_feature_major(
                                nc, lnp, lnsb, consts, load_x,
                                lambda dt, tci: ln1T[:, dt,
                                                     tci * LCH:(tci + 1) * LCH],
                                DT, N, LCH, post_cb=kv_for_chunk,
                                bscale_row=(cl_row if fp8_kv else None))
                    nc.leave_named_scope("phA_ln1", scopeA[0], False)

                    # ----- Phase B remainder: K2/V2 (context) -----
                    scopeB = nc.enter_named_scope("phB_qkv", False)
                    with tc.tile_pool(name="wb", bufs=1) as wpool, \
                         tc.tile_pool(name="projps2", bufs=2, space="PSUM") as pp:
                        # K2/V2 from context (independent of x)
                        ctx_sb = wpool.tile([P, CT, CN], BF16, tag="ctx", bufs=1)
                        for ct in range(CT):
                            nc.sync.dma_start(ctx_sb[:, ct, :], ctxT_v[ct])
                        wk2_sb = wpool.tile([P, CT, D], BF16, tag="w15", bufs=2)
                        nc.sync.dma_start(wk2_sb, wview(wk2T))
                        for oc in range(DT):
                            k_ps = pp.tile([P, CN], F32, tag="ppsm", bufs=2)
                            for it in range(CT):
                                nc.tensor.matmul(
                                    k_ps, wk2_sb[:, it, oc * P:(oc + 1) * P],
                                    ctx_sb[:, it, :],
                                    start=(it == 0), stop=(it == CT - 1))
                            nc.scalar.copy(out=K2_sb[:, oc, :], in_=k_ps)
                        wv2_sb = wpool.tile([P, CT, D], BF16, tag="w15", bufs=2)
                        nc.sync.dma_start(wv2_sb, wview(wv2T))
                        for hc in range(2):
                            v_ps = pp.tile([CN, 512], F32, tag="ppsm", bufs=2)
                            for it in range(CT):
                                nc.tensor.matmul(
                                    v_ps, ctx_sb[:, it, :],
                                    wv2_sb[:, it, hc * 512:(hc + 1) * 512],
                                    start=(it == 0), stop=(it == CT - 1))
                            nc.scalar.copy(
                                out=V2_sb[0:CN, hc * 8:(hc + 1) * 8, :],
                                in_=v_ps.rearrange("p (h d) -> p h d", d=64))
                    nc.leave_named_scope("phB_qkv", scopeB[0], False)

                # ----- Phase C: self-attention heads -----
                scopeC = nc.enter_named_scope("phC_attn", False)
                # prefetch next-phase weights under the attention stream
                wo1_sb = wmain.tile([P, DT, D], BF16, tag="w2m", bufs=2)
                nc.sync.dma_start(wo1_sb, wview(wo1T))
                wq2_sb = wmain.tile([P, DT, D], BF16, tag="w2m", bufs=2)
                nc.sync.dma_start(wq2_sb, wview(wq2T))
                with tc.tile_pool(name="aps", bufs=1, space="PSUM") as apsum, \
                     tc.tile_pool(name="asb", bufs=1) as asb:
                    for h in range(H):
                        j, r0 = h >> 1, (h & 1) * 64
                        o_ps = apsum.tile([65, TO], F32, tag="o_ps", bufs=2)
                        e_tiles = []

                        def av_block(g, o_ps=o_ps, h=h, e_tiles=e_tiles):
                            for i2 in range(2):
                                kt = g * 4 + 2 * i2
                                nc.tensor.matmul(
                                    o_ps, V_sb[:, kt:kt + 2, h, :],
                                    e_tiles[g][:, 2 * i2:2 * i2 + 2, :],
                                    perf_mode=DRow,
                                    start=(kt == 0), stop=(kt == NKT - 2))

                        for g in range(4):
                            # 4 key-tiles share one PSUM tile; one exp each
                            s_all = apsum.tile([P, 4, TO], F32, tag="s_all",
                                               bufs=2)
                            for q in range(4):
                                kt = g * 4 + q
                                # start=True on the first matmul touching
                                # each 2KB PSUM bank (pending-zero is
                                # bank-granular); never re-start a bank.
                                nc.tensor.matmul(
                                    s_all[:, q, :],
                                    K_sb[r0:r0 + 64, j, kt * P:(kt + 1) * P],
                                    Q_sb[r0:r0 + 64, j, :],
                                    start=(q % 2 == 0), stop=True,
                                    skip_group_check=(q > 0))
                            e_all = asb.tile([P, 4, TO], FP8, tag="e_all",
                                             bufs=3)
                            nc.scalar.activation(e_all, s_all, AF.Exp,
                                                 scale=SCALE)
                            e_tiles.append(e_all)
                            # AV for the previous tile group: its exp ran
                            # while this group's scores streamed, so the PE
                            # FIFO never parks on an exp wait.
                            if g >= 1:
                                av_block(g - 1)
                        av_block(3)
                        r_sb = asb.tile([1, TO], F32R, tag="r_sb", bufs=4)
                        with nc.allow_low_precision("f32r == f32 bits"):
                            nc.vector.reciprocal(r_sb, o_ps[64:65, :])
                        r_ps = apsum.tile([64, TO], F32, tag="r_ps", bufs=2)
                        nc.tensor.matmul(r_ps, invav_row[:, :64], r_sb,
                                         start=True, stop=True)
                        r_bc = asb.tile([64, TO], F32, tag="r_bc", bufs=3)
                        nc.vector.tensor_copy(out=r_bc, in_=r_ps)
                        nc.vector.tensor_tensor(out=O_sb[r0:r0 + 64, j, :],
                                                in0=o_ps[0:64, :],
                                                in1=r_bc, op=OP.mult)
                nc.leave_named_scope("phC_attn", scopeC[0], False)

                # ----- Phase D: attn1 out-proj + residual -----
                scopeD = nc.enter_named_scope("phD_oproj", False)
                with tc.tile_pool(name="dps", bufs=3, space="PSUM") as pp:
                    def add_residual(oc, ps):
                        nc.vector.tensor_tensor(
                            out=x_ownT[:, oc, :],
                            in0=x_ownT[:, oc, :].bitcast(F32),
                            in1=ps, op=OP.add)

                    proj_feature_major(pp, wo1_sb, O_sb, add_residual, DT, TO,
                                       bias=bias_sb.get("bo1"))
                nc.leave_named_scope("phD_oproj", scopeD[0], False)

            # ========== attn2 scope: phase E ==========
            scopeE = nc.enter_named_scope("phE_xattn", False)
            with tc.tile_pool(name="ce", bufs=1) as ce:
                ln2T = ce.tile([P, DT, TO], BF16)
                Q2_sb = ce.tile([P, DT, TO], BF16)
                O2_sb = ce.tile([P, DT, TO], BF16)

                with tc.tile_pool(name="lnps2", bufs=2, space="PSUM") as lnp, \
                     tc.tile_pool(name="lnsb2", bufs=2) as lnsb:
                    _ln_feature_major(
                        nc, lnp, lnsb, consts,
                        lambda dt, tci: x_ownT[:, dt, :],
                        lambda dt, tci: ln2T[:, dt, :],
                        DT, TO, TO, alt_mult=True)

                wo2_sb = wmain.tile([P, DT, D], BF16, tag="w2m", bufs=2)
                with tc.tile_pool(name="eps_", bufs=2, space="PSUM") as pp:
                    proj_feature_major(
                        pp, wq2_sb, ln2T,
                        lambda oc, ps: nc.scalar.copy(out=Q2_sb[:, oc, :],
                                                      in_=ps),
                        DT, TO, bias=bias_sb.get("bq2"))
                    nc.sync.dma_start(wo2_sb, wview(wo2T))

                with tc.tile_pool(name="aps2", bufs=1, space="PSUM") as apsum, \
                     tc.tile_pool(name="asb2", bufs=1) as asb:
                    for grp in range(4):
                        den_ps = apsum.tile([P, 2, 4], F32, tag="den", bufs=1)
                        o_all = apsum.tile([64, 4, TO], F32, tag="o_all",
                                           bufs=1)
                        e_list = []
                        s4 = apsum.tile([CN, 4, TO], F32, tag="s_ps", bufs=1)
                        for h8 in range(4):
                            h = grp * 4 + h8
                            j, r0 = h >> 1, (h & 1) * 64
                            nc.tensor.matmul(
                                s4[:, h8, :], K2_sb[r0:r0 + 64, j, :],
                                Q2_sb[r0:r0 + 64, j, :],
                                start=(h8 % 2 == 0), stop=True,
                                skip_group_check=(h8 > 0))
                            e_t = asb.tile([CN, TO], BF16, tag="e_t", bufs=8)
                            nc.scalar.activation(e_t, s4[:, h8, :], AF.Exp,
                                                 scale=SCALE)
                            e_list.append(e_t)
                        for h8 in range(4):
                            h = grp * 4 + h8
                            e_t = e_list[h8]
                            # [64, 4, 256] f32 = 2 banks: h8 0/1 in bank 0,
                            # h8 2/3 in bank 1 -> start on h8 0 and 2 only.
                            nc.tensor.matmul(o_all[:, h8, :], V2_sb[0:CN, h, :],
                                             e_t, start=(h8 % 2 == 0),
                                             stop=True,
                                             skip_group_check=(h8 > 0))
                            # denominators: den[q, c, h8] = sum_k e[k, q]
                            for c in range(2):
                                nc.tensor.matmul(
                                    den_ps[:, c, h8:h8 + 1],
                                    e_t[:, c * P:(c + 1) * P],
                                    ones_colb[0:CN, :],
                                    start=(h8 == 0 and c == 0), stop=True,
                                    skip_group_check=(h8 > 0 or c > 0))
                        rf = asb.tile([P, 2, 4], BF16, tag="rf", bufs=2)
                        with nc.allow_low_precision("softmax denom to bf16"):
                            nc.vector.reciprocal(rf, den_ps)
                        rT_ps = apsum.tile([8, P], BF16, tag="rT", bufs=1)
                        nc.tensor.transpose(rT_ps, rf.rearrange("p c h -> p (c h)"),
                                            ident_sb)
                        rT_sb = asb.tile([8, P], BF16, tag="rTs", bufs=2)
                        nc.vector.tensor_copy(out=rT_sb, in_=rT_ps)
                        for h8 in range(4):
                            h = grp * 4 + h8
                            j, r0 = h >> 1, (h & 1) * 64
                            r_ps = apsum.tile([64, 2, P], F32, tag="r_ps",
                                              bufs=1)
                            for c in range(2):
                                idx = c * 4 + h8
                                nc.tensor.matmul(
                                    r_ps[:, c, :],
                                    sel_sb[0:8, idx * 64:(idx + 1) * 64],
                                    rT_sb, start=(c == 0), stop=True,
                                    skip_group_check=(c > 0))
                            r_bc = asb.tile([64, TO], F32, tag="r_bc", bufs=3)
                            nc.scalar.copy(
                                out=r_bc.rearrange("p (c q) -> p c q", c=2),
                                in_=r_ps)
                            nc.vector.tensor_tensor(
                                out=O2_sb[r0:r0 + 64, j, :],
                                in0=o_all[:, h8, :], in1=r_bc, op=OP.mult)

                with tc.tile_pool(name="eps2", bufs=3, space="PSUM") as pp:
                    def add_residual2(oc, ps):
                        nc.vector.tensor_tensor(
                            out=x_ownT[:, oc, :],
                            in0=x_ownT[:, oc, :].bitcast(F32),
                            in1=ps, op=OP.add)

                    proj_feature_major(pp, wo2_sb, O2_sb, add_residual2, DT, TO,
                                       bias=bias_sb.get("bo2"))
            nc.leave_named_scope("phE_xattn", scopeE[0], False)

            # ========== FFN scope: phase F ==========
            scopeF = nc.enter_named_scope("phF_ffn", False)
            with tc.tile_pool(name="cf", bufs=1) as cf:
                ln3T = cf.tile([P, DT, TO], BF16)
                Hbuf = cf.tile([P, FT, TO], BF16)

                wgT_v = wview(wgT)
                wfT_v = wfT.rearrange("(f p) o -> f p o", p=P)
                bgeg = bias_sb.get("bgeg")
                bff = bias_sb.get("bff")
                # interleave GEGLU groups with ffout blocks: per fg (8 f-tiles)
                # compute Hbuf then immediately contract into the output.
                with tc.tile_pool(name="wg", bufs=1) as wgpool, \
                     tc.tile_pool(name="wfp", bufs=1) as wfpool:

                    def load_wg(g):
                        wg_h = wgpool.tile([P, DT, 512], BF16, tag="wgh",
                                           bufs=2)
                        nc.sync.dma_start(
                            wg_h, wgT_v[:, :, g * 512:(g + 1) * 512])
                        wg_g = wgpool.tile([P, DT, 512], BF16, tag="wgg",
                                           bufs=2)
                        nc.sync.dma_start(
                            wg_g,
                            wgT_v[:, :, FF + g * 512:FF + (g + 1) * 512])
                        return wg_h, wg_g

                    def load_wf(fg):
                        tiles = []
                        for f8 in range(8):
                            wt = wfpool.tile([P, D], BF16, tag="wft", bufs=16)
                            nc.sync.dma_start(wt, wfT_v[fg * 8 + f8])
                            tiles.append(wt)
                        return tiles

                    # first weight groups prefetch under the LN3 compute so
                    # the PE has work the moment LN3 lands
                    wg0 = load_wg(0)
                    wf0 = load_wf(0)
                    with tc.tile_pool(name="lnps3", bufs=2,
                                      space="PSUM") as lnp, \
                         tc.tile_pool(name="lnsb3", bufs=2) as lnsb:
                        _ln_feature_major(
                            nc, lnp, lnsb, consts,
                            lambda dt, tci: x_ownT[:, dt, :],
                            lambda dt, tci: ln3T[:, dt, :],
                            DT, TO, TO, alt_mult=True)

                    ffn_pools = tc.tile_pool(name="gps", bufs=1, space="PSUM")
                    gpsum = ffn_pools.__enter__()
                    yp_pools = tc.tile_pool(name="yps", bufs=2, space="PSUM")
                    yp_ = yp_pools.__enter__()
                    gsb_pools = tc.tile_pool(name="gsb", bufs=3)
                    gsb = gsb_pools.__enter__()
                    for fg in range(4):
                        wf_tiles = wf0 if fg == 0 else load_wf(fg)
                        for g2 in range(2):
                            g = fg * 2 + g2
                            wg_h, wg_g = wg0 if g == 0 else load_wg(g)
                            for fi in range(4):
                                f = g * 4 + fi
                                h_ps = gpsum.tile([P, TO], F32, tag="h_ps",
                                                  bufs=2)
                                for it in range(DT):
                                    nc.tensor.matmul(
                                        h_ps, wg_h[:, it, fi * P:(fi + 1) * P],
                                        ln3T[:, it, :],
                                        start=(it == 0),
                                        stop=(it == DT - 1 and bgeg is None))
                                if bgeg is not None:
                                    nc.tensor.matmul(
                                        h_ps, bgeg[:, f * P:(f + 1) * P],
                                        ones_b[:, :TO], start=False, stop=True)
                                g_ps = gpsum.tile([P, TO], F32, tag="g_ps",
                                                  bufs=2)
                                for it in range(DT):
                                    nc.tensor.matmul(
                                        g_ps, wg_g[:, it, fi * P:(fi + 1) * P],
                                        ln3T[:, it, :],
                                        start=(it == 0),
                                        stop=(it == DT - 1 and bgeg is None))
                                if bgeg is not None:
                                    nc.tensor.matmul(
                                        g_ps,
                                        bgeg[:, FF + f * P:FF + (f + 1) * P],
                                        ones_b[:, :TO], start=False, stop=True)
                                gel = gsb.tile([P, TO], F32, tag="gel", bufs=3)
                                nc.scalar.activation(gel, g_ps, AF.Gelu)
                                nc.vector.tensor_tensor(out=Hbuf[:, f, :],
                                                        in0=h_ps, in1=gel,
                                                        op=OP.mult)
                        # ffout for this fg block (two-level accumulation;
                        # spills add into x_ownT)
                        for oc in range(DT):
                            i_ps = yp_.tile([P, TO], F32, tag="i_ps")
                            add_bias = bff is not None and fg == 3
                            for f8 in range(8):
                                nc.tensor.matmul(
                                    i_ps, wf_tiles[f8][:, oc * P:(oc + 1) * P],
                                    Hbuf[:, fg * 8 + f8, :],
                                    start=(f8 == 0),
                                    stop=(f8 == 7 and not add_bias))
                            if add_bias:
                                nc.tensor.matmul(
                                    i_ps, bff[:, oc * P:(oc + 1) * P],
                                    ones_b[:, :TO], start=False, stop=True)
                            nc.vector.tensor_tensor(
                                out=x_ownT[:, oc, :],
                                in0=x_ownT[:, oc, :].bitcast(F32),
                                in1=i_ps, op=OP.add)
                            if fg == 3:
                                nc.sync.dma_start(yT_v[:, oc, :],
                                                  x_ownT[:, oc, :])
                    gsb_pools.__exit__(None, None, None)
                    yp_pools.__exit__(None, None, None)
                    ffn_pools.__exit__(None, None, None)
            nc.leave_named_scope("phF_ffn", scopeF[0], False)

    nc.finalize()
    return nc


_CACHE = {}


def kernel(**inputs):
    def f32c(a):
        return np.ascontiguousarray(np.asarray(a, dtype=np.float32))

    def bfT(w):
        """W [out,in] (optionally gain-folded) -> bf16 W.T contiguous."""
        return np.ascontiguousarray(w.T).astype(ml_dtypes.bfloat16)

    x = f32c(inputs["hidden_states"])[0]          # [N, D]
    ctx = f32c(inputs["context"])[0]              # [CN, CD]
    g1 = f32c(inputs["ln1_g"]); b1 = f32c(inputs["ln1_b"])
    g2 = f32c(inputs["ln2_g"]); b2 = f32c(inputs["ln2_b"])
    g3 = f32c(inputs["ln3_g"]); b3 = f32c(inputs["ln3_b"])
    wq1 = f32c(inputs["wq1"]); wk1 = f32c(inputs["wk1"]); wv1 = f32c(inputs["wv1"])
    wo1 = f32c(inputs["wo1"]); bo1 = f32c(inputs["bo1"])
    wq2 = f32c(inputs["wq2"]); wk2 = f32c(inputs["wk2"]); wv2 = f32c(inputs["wv2"])
    wo2 = f32c(inputs["wo2"]); bo2 = f32c(inputs["bo2"])
    wg = f32c(inputs["w_geglu"]); bg = f32c(inputs["b_geglu"])
    wf = f32c(inputs["w_ffout"]); bf = f32c(inputs["b_ffout"])

    bq1 = wq1 @ b1; bk1 = wk1 @ b1; bv1 = wv1 @ b1
    bq2 = wq2 @ b2
    bgeg = bg + wg @ b3
    flags = (bool(np.any(bq1) or np.any(bk1) or np.any(bv1)), bool(np.any(bo1)),
             bool(np.any(bq2)), bool(np.any(bo2)), bool(np.any(bgeg)),
             bool(np.any(bf)))

    def pow2scale(w):
        a = float(np.abs(w).max())
        if a <= 0:
            return 1.0
        return float(2.0 ** int(np.floor(np.log2(224.0 / a))))

    wq1g = wq1 * g1[None, :]
    wk1g = wk1 * g1[None, :]
    wv1g = wv1 * g1[None, :]
    fp8_kv = not flags[0]
    qexps = (pow2scale(wq1g), pow2scale(wk1g),
             pow2scale(wv1g)) if fp8_kv else None

    key = (flags, qexps)
    if key not in _CACHE:
        _CACHE[key] = build(flags, qexps)
    nc = _CACHE[key]

    xT = np.ascontiguousarray(x.T)                # [D, N]
    bf16 = ml_dtypes.bfloat16
    fp8 = ml_dtypes.float8_e4m3

    def f8T(w, s):
        return np.ascontiguousarray(w.T * s).astype(fp8)

    selm = np.zeros((16, 1024), np.float32)
    for r in range(16):
        selm[r, r * 64:(r + 1) * 64] = 1.0
    shared = {
        "ctxT": np.ascontiguousarray(ctx.T).astype(bf16),
        "wq1T": f8T(wq1g, qexps[0]) if fp8_kv else bfT(wq1g),
        "wk1T": f8T(wk1g, qexps[1]) if fp8_kv else bfT(wk1g),
        "wv1T": f8T(wv1g, qexps[2]) if fp8_kv else bfT(wv1g),
        "wo1T": bfT(wo1),
        "wq2T": bfT(wq2 * g2[None, :]),
        "wk2T": bfT(wk2),
        "wv2T": bfT(wv2),
        "wo2T": bfT(wo2),
        "wgT": bfT(wg * g3[None, :]),
        "wfT": bfT(wf),
        "onesc": np.ones((P, 1), np.float32),
        "onescb": np.ones((P, 1), bf16),
        "onesr": np.ones((1, P), np.float32),
        "invavr": np.full((1, P), 1.0 / AVS, np.float32),
        "onesb": np.ones((1, 512), bf16),
        "selm": selm.astype(bf16),
        "identb": np.eye(P, dtype=np.float32).astype(bf16),
    }
    if flags[0]:
        shared["bq1"] = bq1[None, :].astype(bf16)
        shared["bk1"] = bk1[None, :].astype(bf16)
        shared["bv1"] = bv1[None, :].astype(bf16)
    if flags[1]:
        shared["bo1"] = bo1[None, :].astype(bf16)
    if flags[2]:
        shared["bq2"] = bq2[None, :].astype(bf16)
    if flags[3]:
        shared["bo2"] = bo2[None, :].astype(bf16)
    if flags[4]:
        shared["bgeg"] = bgeg[None, :].astype(bf16)
    if flags[5]:
        shared["bff"] = bf[None, :].astype(bf16)

    in_maps = []
    for c in range(NCORES):
        m = dict(shared)
        # rotate so core c's own tokens occupy columns 0:TO
        m["xT"] = np.ascontiguousarray(np.roll(xT, -c * TO, axis=1))
        in_maps.append(m)

    res = run_bass_kernel_spmd(nc, in_maps, core_ids=list(range(NCORES)))
    yT = np.concatenate([r["yT"] for r in res.results], axis=1)  # [D, N]
    return np.ascontiguousarray(yT.T)[None].astype(np.float32)
